# revision 1
# baseline (speedup 1.0000x reference)
"""Multi-head self-attention TRN2 Bass kernel.

Problem: x[2, 2048, 1024], 16 heads x 64 dim, fp32.
Sharding: 8 cores = 2 batches x 4 head-groups (4 heads each).
Each core computes its batch's partial output (its 4 heads through
QKV -> attention -> output projection rows); host sums the 4 partials
per batch and adds bo.

Per-core structure (avoids every attention transpose):
  - x^T pre-tiled on host (bf16) and loaded as 4 contiguous 1MB DMAs,
    chained so chunk 0 lands first at full bandwidth.
  - q^T, k^T [256, 2048] bf16  (head h at partitions (h%2)*64 of tile h//2)
  - V' [2048, 4, 65] bf16  (per head: V columns + a ones column)
  - scores computed TRANSPOSED: S^T[k,q] = k^T.T @ q^T as row-tiled
    head-PAIRS (two concurrent K=64 matmuls); 1/sqrt(hd) folded into
    Wq/bq on host.
  - exp on ACT -> A^T bf16 (rolling 4-deep buffer), directly the moving
    operand of out^T[65, q] = V'^T @ A^T; row 64 = softmax row sums
    (ones-column trick).
  - normalize: fast-reciprocal on DVE; the [1,512] -> [64,512] partition
    broadcast is a rank-1 PE outer product (ones[1,64].T @ recip).
  - out_proj: head pairs stacked to K=128, emitted two q-tiles per pair
    boundary so it never stalls the in-order PE stream.
  - the ACT exp stream is the bottleneck (~139us busy); the pre-exp
    critical path carries only kT m=0 + V + qT chunk 0, while kT m=1 and
    qT chunks 1-3 drain through the attention loop's slack slots. Dummy
    matmuls fill residual PE bubbles to keep the HAM clock-gate warm at
    2.4 GHz (cold phases run the PE at 1.2 GHz).
"""

import numpy as np

S = 2048          # sequence length per batch
H = 1024          # hidden
G = 256           # head-group width (4 heads x 64)
HD = 65           # V' columns per head (64 + ones)
NHL = 4           # heads per core
N_CORES = 8

_CACHE = {}


def _build():
    if "nc" in _CACHE:
        return _CACHE["nc"]

    import concourse.bass as bass
    import concourse.mybir as mybir
    import concourse.tile as tile
    from concourse import bacc
    from concourse.tile_rust import add_dep_helper

    f32 = mybir.dt.float32
    bf16 = mybir.dt.bfloat16
    EXP = mybir.ActivationFunctionType.Exp

    nc = bacc.Bacc("TRN2", target_bir_lowering=False, debug=False,
                   num_devices=N_CORES)

    xt_in = nc.dram_tensor("xt", [4, 128, 8, 512], bf16, kind="ExternalInput")
    wq_in = nc.dram_tensor("wq", [H, G], bf16, kind="ExternalInput")
    wk_in = nc.dram_tensor("wk", [H, G], bf16, kind="ExternalInput")
    wv_in = nc.dram_tensor("wv", [H, G], bf16, kind="ExternalInput")
    bq_in = nc.dram_tensor("bq", [G, 1], f32, kind="ExternalInput")
    bk_in = nc.dram_tensor("bk", [G, 1], f32, kind="ExternalInput")
    bv_in = nc.dram_tensor("bv", [G], f32, kind="ExternalInput")
    wo_in = nc.dram_tensor("wo", [NHL, 64, H], bf16, kind="ExternalInput")
    out_d = nc.dram_tensor("out", [S, H], f32, kind="ExternalOutput")

    with tile.TileContext(nc) as tc:
        with tc.tile_pool(name="persist", bufs=1) as persist:
            qT = persist.tile([128, 2, S], bf16)     # [qd, m, s]
            kT = persist.tile([128, 2, S], bf16)
            vp = persist.tile([128, 16, NHL, HD], bf16)  # [s-part, st, h, col]
            bq_sb = persist.tile([128, 2, 1], f32)
            bk_sb = persist.tile([128, 2, 1], f32)
            bv_bc = persist.tile([128, G], f32)
            wo_pr = persist.tile([128, 2, H], bf16)
            ones64 = persist.tile([1, 64], bf16)

            # ------- Phase A + q-chunk-0 scores/exp overlap -------
            w_pool = persist
            xT_pool = persist
            with (
                tc.tile_pool(name="ps_a", bufs=2, space="PSUM") as ps_a,
            ):
                wq_sb = w_pool.tile([128, 8, G], bf16)
                wk_sb = w_pool.tile([128, 8, G], bf16)
                wv_sb = w_pool.tile([128, 8, G], bf16)
                nc.sync.dma_start(
                    out=wq_sb, in_=wq_in.ap().rearrange("(t p) d -> p t d", p=128))
                nc.sync.dma_start(
                    out=wk_sb, in_=wk_in.ap().rearrange("(t p) d -> p t d", p=128))
                nc.sync.dma_start(
                    out=wv_sb, in_=wv_in.ap().rearrange("(t p) d -> p t d", p=128))

                # per-chunk x^T tiles; host pre-tiles x^T into exactly
                # this layout so each chunk is one contiguous 1MB DMA
                xTc = [xT_pool.tile([128, 8, 512], bf16, name=f"xT_{jc}")
                       for jc in range(4)]
                x_dmas = [nc.sync.dma_start(out=xTc[jc], in_=xt_in.ap()[jc])
                          for jc in range(4)]
                for jc in range(1, 4):
                    # chain the chunk loads so chunk 0 lands first at full
                    # bandwidth instead of round-robin across all four
                    add_dep_helper(x_dmas[jc].ins, x_dmas[jc - 1].ins,
                                   reason="serialize x chunk loads")

                nc.sync.dma_start(
                    out=bq_sb, in_=bq_in.ap().rearrange("(m p) o -> p m o", p=128))
                nc.sync.dma_start(
                    out=bk_sb, in_=bk_in.ap().rearrange("(m p) o -> p m o", p=128))
                # broadcast bv along partitions (stride-0 partition AP)
                bv_ap = bass.AP(tensor=bv_in, offset=0, ap=[[0, 128], [1, G]])
                nc.gpsimd.dma_start(out=bv_bc, in_=bv_ap)
                # Wo as stacked head pairs: [two*64+p, pr, n]
                nc.sync.dma_start(
                    out=wo_pr,
                    in_=wo_in.ap().rearrange("(pr two) p n -> (two p) pr n", two=2))
                # ones columns of V'
                nc.gpsimd.memset(vp[:, :, :, 64:65], 1.0)
                nc.gpsimd.memset(ones64, 1.0)

                def dummy_a(n=512):
                    ps_d = ps_a.tile([128, 512], f32, tag="dum", bufs=1)
                    nc.tensor.matmul(
                        ps_d[:, 0:n], lhsT=wq_sb[:, 0, 0:128],
                        rhs=wq_sb[:, 0:2, :].rearrange("p a b -> p (a b)")[:, 0:n],
                        start=True, stop=True)

                def qk_half(w_sb, b_sb, dst, jc, m, half, st):
                    sl = slice(jc * 512, (jc + 1) * 512)
                    if half == 0:
                        st["ps"] = ps_a.tile([128, 512], f32, tag="qk",
                                             name=f"psq_{id(w_sb)}_{jc}_{m}")
                    for ht in range(half * 4, half * 4 + 4):
                        mm = nc.tensor.matmul(
                            st["ps"],
                            lhsT=w_sb[:, ht, m * 128:(m + 1) * 128],
                            rhs=xTc[jc][:, ht, :],
                            start=(ht == 0), stop=(ht == 7))
                        st.setdefault("first_mm", mm)
                    if half == 1:
                        nc.vector.tensor_scalar_add(
                            dst[:, m, sl], st["ps"], b_sb[:, m, :])

                def v_unit(st16):
                    ps_vt = ps_a.tile([128, 512], f32, tag="qk",
                                      name=f"psv_{st16}")
                    for ht in range(8):
                        nc.tensor.matmul(
                            ps_vt[:, 0:G],
                            lhsT=xTc[st16 // 4][:, ht,
                                                (st16 % 4) * 128:
                                                (st16 % 4 + 1) * 128],
                            rhs=wv_sb[:, ht, :],
                            start=(ht == 0), stop=(ht == 7))
                    nc.vector.tensor_add(
                        vp[:, st16, :, 0:64],
                        ps_vt[:, 0:G].rearrange("p (h d) -> p h d", h=NHL),
                        bv_bc.rearrange("p (h d) -> p h d", h=NHL))

                # minimal pre-exp critical path: kT m=0 + V (needed by the
                # attn@V interleave) + qT chunk 0. kT m=1 and qT chunks 1-3
                # are deferred into the attention stream's slack slots.
                for _ in range(10):
                    dummy_a()
                for jc in range(4):
                    st = {}
                    qk_half(wk_sb, bk_sb, kT, jc, 0, 0, st)
                    qk_half(wk_sb, bk_sb, kT, jc, 0, 1, st)
                    for i in range(4):
                        v_unit(jc * 4 + i)
                for m in range(2):
                    st = {}
                    qk_half(wq_sb, bq_sb, qT, 0, m, 0, st)
                    qk_half(wq_sb, bq_sb, qT, 0, m, 1, st)
            # ---------------- Phase B: attention + out_proj ----------------
            with (
                tc.tile_pool(name="at_roll", bufs=2) as at_pool,
                tc.tile_pool(name="outP", bufs=4) as op_pool,
                tc.tile_pool(name="tmpo", bufs=1) as tmpo_pool,
                tc.tile_pool(name="sums", bufs=4) as sums_pool,
                tc.tile_pool(name="osb", bufs=2) as osb_pool,
                tc.tile_pool(name="ps_s", bufs=2, space="PSUM") as ps_s_pool,
                tc.tile_pool(name="ps_av", bufs=2, space="PSUM") as ps_av_pool,
                tc.tile_pool(name="ps_op", bufs=1, space="PSUM") as ps_op_pool,
            ):
                def dummy(n):
                    ps_d = ps_op_pool.tile([128, 512], f32, tag="dummy")
                    nc.tensor.matmul(ps_d[:, 0:n], lhsT=kT[:, 0, 0:128],
                                     rhs=qT[:, 0, 0:n], start=True, stop=True)

                def norm_head(outP, ps_av, hh, qc, mt):
                    # evacuate PSUM right away to release the bank; run the
                    # normalize chain from SBUF
                    uout = tmpo_pool.tile([HD, 512], f32, tag="uout",
                                          name=f"uo_{qc}_{mt}_{hh}", bufs=4)
                    nc.vector.tensor_copy(uout, ps_av)
                    sums = sums_pool.tile([1, 512], f32, tag="sums",
                                          name=f"sm_{qc}_{mt}_{hh}")
                    nc.vector.tensor_copy(sums, uout[64:65, :])
                    recip = sums_pool.tile([1, 512], f32, tag="recip",
                                           name=f"rc_{qc}_{mt}_{hh}")
                    nc.vector.reciprocal_approx_fast(out=recip, in_=sums)
                    recip_bf = sums_pool.tile([1, 512], bf16, tag="recipb",
                                              name=f"rcb_{qc}_{mt}_{hh}")
                    nc.vector.tensor_copy(recip_bf, recip)
                    # broadcast along partitions: rank-1 outer product on
                    # the PE (ones[1,64].T @ recip[1,512] -> [64,512])
                    rbc = ps_op_pool.tile([64, 512], f32, tag="dummy",
                                          name=f"rb_{qc}_{mt}_{hh}")
                    nc.tensor.matmul(rbc, lhsT=ones64, rhs=recip_bf,
                                     start=True, stop=True)
                    nc.vector.tensor_mul(
                        outP[hh * 64:hh * 64 + 64, :], uout[0:64, :], rbc)

                fillers = []

                def fill_qk_half(w_sb, b_sb, dst, jc, m, half, st):
                    sl = slice(jc * 512, (jc + 1) * 512)
                    if half == 0:
                        st["ps"] = ps_op_pool.tile(
                            [128, 512], f32, tag="dummy",
                            name=f"psf_{id(w_sb)}_{jc}_{m}")
                    for ht in range(half * 4, half * 4 + 4):
                        nc.tensor.matmul(
                            st["ps"],
                            lhsT=w_sb[:, ht, m * 128:(m + 1) * 128],
                            rhs=xTc[jc][:, ht, :],
                            start=(ht == 0), stop=(ht == 7))
                    if half == 1:
                        nc.vector.tensor_scalar_add(
                            dst[:, m, sl], st["ps"], b_sb[:, m, :])

                def add_fill(w_sb, b_sb, dst, jc, m):
                    st = {}
                    fillers.append(lambda: fill_qk_half(
                        w_sb, b_sb, dst, jc, m, 0, st))
                    fillers.append(lambda: fill_qk_half(
                        w_sb, b_sb, dst, jc, m, 1, st))

                for jc in range(4):
                    add_fill(wk_sb, bk_sb, kT, jc, 1)
                for jc in range(1, 4):
                    for m in range(2):
                        add_fill(wq_sb, bq_sb, qT, jc, m)

                def oproj_unit(qc, outPs, qt, tail=False):
                    # out_proj for one q-tile (K=128 stacked pairs); at the
                    # kernel tail the freed score slots double-buffer it
                    osb = osb_pool.tile([128, H], f32, tag="osb",
                                        name=f"osb_{qc}_{qt}")
                    for ncx in range(2):
                        if tail:
                            ps_op = ps_s_pool.tile(
                                [128, 2, 512], f32, tag="s",
                                name=f"psot_{qc}_{qt}_{ncx}")[:, 0, :]
                        else:
                            ps_op = ps_op_pool.tile(
                                [128, 512], f32, tag="oproj",
                                name=f"pso_{qc}_{qt}_{ncx}")
                        for pr in range(2):
                            nc.tensor.matmul(
                                ps_op,
                                lhsT=outPs[pr][:, qt * 128:(qt + 1) * 128],
                                rhs=wo_pr[:, pr, ncx * 512:(ncx + 1) * 512],
                                start=(pr == 0), stop=(pr == 1))
                        nc.vector.tensor_copy(
                            osb[:, ncx * 512:(ncx + 1) * 512], ps_op)
                    nc.sync.dma_start(
                        out=out_d.ap()[qc * 512 + qt * 128:
                                       qc * 512 + (qt + 1) * 128, :],
                        in_=osb)

                prev = None  # (qc, outPs, next_qt) awaiting out_proj
                for qc in range(4):  # q-chunks of 512
                    qsl = slice(qc * 512, (qc + 1) * 512)
                    outPs = []
                    for mt in range(2):  # head pair (2mt, 2mt+1)
                        attnT = at_pool.tile([128, 2, 4, 512], bf16,
                                             tag="at", name=f"at_{qc}_{mt}")
                        ps_avs = [ps_av_pool.tile([HD, 512], f32, tag="av",
                                                  name=f"av_{qc}_{mt}_{hh}")
                                  for hh in range(2)]
                        for kt in range(16):
                            ps_s = ps_s_pool.tile([128, 2, 512], f32, tag="s")
                            for hh in range(2):
                                nc.tensor.matmul(
                                    ps_s[:, hh, :],
                                    lhsT=kT[hh * 64:hh * 64 + 64, mt,
                                            kt * 128:(kt + 1) * 128],
                                    rhs=qT[hh * 64:hh * 64 + 64, mt, qsl],
                                    start=True, stop=True)
                            nc.scalar.activation(
                                out=attnT[:, :, kt % 4, :], in_=ps_s, func=EXP)
                            for hh in range(2):
                                nc.tensor.matmul(
                                    ps_avs[hh],
                                    lhsT=vp[:, kt, 2 * mt + hh, :],
                                    rhs=attnT[:, hh, kt % 4, :],
                                    start=(kt == 0), stop=(kt == 15))
                            if fillers and kt % 2 == 1:
                                fillers.pop(0)()
                            elif kt % 4 == 0:
                                dummy(256)
                        outP = op_pool.tile([128, 512], bf16, tag="outP",
                                            name=f"outP_{qc}_{mt}")
                        for hh in range(2):
                            norm_head(outP, ps_avs[hh], hh, qc, mt)
                        outPs.append(outP)
                        for _ in range(2):
                            dummy(512)
                        if prev is not None:
                            # two q-tiles of the previous q-chunk's out_proj
                            # at each pair boundary
                            pq, pouts, qt0 = prev
                            oproj_unit(pq, pouts, qt0)
                            oproj_unit(pq, pouts, qt0 + 1)
                            prev = (pq, pouts, qt0 + 2) if qt0 + 2 < 4 else None
                    prev = (qc, outPs, 0)
                # cover the last normalize chain, then final out_proj
                for _ in range(4):
                    dummy(512)
                pq, pouts, qt0 = prev
                for qt in range(qt0, 4):
                    oproj_unit(pq, pouts, qt, tail=True)

    nc.compile()
    _CACHE["nc"] = nc
    return nc


def make_in_maps(x, Wq, bq, Wk, bk, Wv, bv, Wo):
    import ml_dtypes
    bf = ml_dtypes.bfloat16

    x = np.asarray(x, dtype=np.float32)
    Wq = np.asarray(Wq, dtype=np.float32)
    bq = np.asarray(bq, dtype=np.float32)
    Wk = np.asarray(Wk, dtype=np.float32)
    bk = np.asarray(bk, dtype=np.float32)
    Wv = np.asarray(Wv, dtype=np.float32)
    bv = np.asarray(bv, dtype=np.float32)
    Wo = np.asarray(Wo, dtype=np.float32)

    scale = np.float32(1.0 / 8.0)  # 1/sqrt(64)

    in_maps = []
    for core in range(N_CORES):
        b = core // 4
        g = core % 4
        cs = slice(g * G, (g + 1) * G)
        in_maps.append({
            "xt": np.ascontiguousarray(
                x[b].reshape(4, 512, 8, 128).transpose(0, 3, 2, 1)).astype(bf),
            "wq": np.ascontiguousarray(Wq[:, cs] * scale).astype(bf),
            "wk": np.ascontiguousarray(Wk[:, cs]).astype(bf),
            "wv": np.ascontiguousarray(Wv[:, cs]).astype(bf),
            "bq": np.ascontiguousarray((bq[cs] * scale).reshape(G, 1)),
            "bk": np.ascontiguousarray(bk[cs].reshape(G, 1)),
            "bv": np.ascontiguousarray(bv[cs]),
            "wo": np.ascontiguousarray(Wo[cs, :].reshape(NHL, 64, H)).astype(bf),
        })
    return in_maps


def kernel(x, Wq, bq, Wk, bk, Wv, bv, Wo, bo):
    from concourse.bass_utils import run_bass_kernel_spmd

    bo = np.asarray(bo, dtype=np.float32)
    nc = _build()
    in_maps = make_in_maps(x, Wq, bq, Wk, bk, Wv, bv, Wo)
    res = run_bass_kernel_spmd(nc, in_maps, core_ids=list(range(N_CORES)))

    out = np.empty((2, S, H), dtype=np.float32)
    for b in range(2):
        acc = res.results[4 * b]["out"].astype(np.float32)
        for g in range(1, 4):
            acc = acc + res.results[4 * b + g]["out"]
        out[b] = acc + bo
    return out



# revision 9
# speedup vs baseline: 1.0175x; 1.0175x over previous
"""Multi-head self-attention TRN2 Bass kernel.

Problem: x[2, 2048, 1024], 16 heads x 64 dim, fp32.
Sharding: 8 cores = 2 batches x 4 head-groups (4 heads each).
Each core computes its batch's partial output (its 4 heads through
QKV -> attention -> output projection rows); host sums the 4 partials
per batch and adds bo.

Single fully-pipelined stream (no separate projection phase):
  - warmup matmuls on a memset tile from t~0 keep the PE p-state/HAM
    clock ramping while the input DMAs land (~9-12us lead-in).
  - minimal prologue: kT m=0 seq-chunk 0 + qT chunk 0 m=0, then the
    attention master loop starts immediately (~13us vs ~49us before).
  - ALL remaining projection work (K m0 jc1-3, K m1, Q c0 m1, Q c1-3,
    V st0-15) runs as deadline-scheduled fillers inside the attention
    stream: forced just-in-time by data deadlines, plus linear pacing
    so the late (ACT-bound) units carry real work instead of dummies.
  - scores computed TRANSPOSED per head-pair (two K=64 matmuls); exp on
    ACT -> A^T bf16 rolling buffer; AV lagged one kt slot behind exp so
    the PE never waits on the ACT engine.
  - normalize via fast-reciprocal + rank-1 PE broadcast; out_proj
    (K=128 stacked head-pairs) enqueued as fillers into the next unit.
  - output partials DMA'd out as bf16 (halves output traffic; host
    accumulates in f32 and adds bo).
"""

import numpy as np

S = 2048          # sequence length per batch
H = 1024          # hidden
G = 256           # head-group width (4 heads x 64)
HD = 65           # V' columns per head (64 + ones)
NHL = 4           # heads per core
N_CORES = 8

_CACHE = {}


def _build():
    if "nc" in _CACHE:
        return _CACHE["nc"]

    import concourse.bass as bass
    import concourse.mybir as mybir
    import concourse.tile as tile
    from concourse import bacc
    from concourse.tile_rust import add_dep_helper

    f32 = mybir.dt.float32
    bf16 = mybir.dt.bfloat16
    EXP = mybir.ActivationFunctionType.Exp

    nc = bacc.Bacc("TRN2", target_bir_lowering=False, debug=False,
                   num_devices=N_CORES)

    xt_in = nc.dram_tensor("xt", [4, 128, 8, 512], bf16, kind="ExternalInput")
    wq_in = nc.dram_tensor("wq", [H, G], bf16, kind="ExternalInput")
    wk_in = nc.dram_tensor("wk", [H, G], bf16, kind="ExternalInput")
    wv_in = nc.dram_tensor("wv", [H, G], bf16, kind="ExternalInput")
    bq_in = nc.dram_tensor("bq", [G, 1], f32, kind="ExternalInput")
    bk_in = nc.dram_tensor("bk", [G, 1], f32, kind="ExternalInput")
    bv_in = nc.dram_tensor("bv", [G], f32, kind="ExternalInput")
    wo_in = nc.dram_tensor("wo", [NHL, 64, H], bf16, kind="ExternalInput")
    out_d = nc.dram_tensor("out", [S, H], bf16, kind="ExternalOutput")

    with tile.TileContext(nc) as tc:
        with (
            tc.tile_pool(name="persist", bufs=1) as persist,
            tc.tile_pool(name="at_roll", bufs=2) as at_pool,
            tc.tile_pool(name="outP", bufs=4) as op_pool,
            tc.tile_pool(name="tmpo", bufs=1) as tmpo_pool,
            tc.tile_pool(name="sums", bufs=4) as sums_pool,
            tc.tile_pool(name="osb", bufs=2) as osb_pool,
            tc.tile_pool(name="ps_s", bufs=2, space="PSUM") as ps_s_pool,
            tc.tile_pool(name="ps_av", bufs=2, space="PSUM") as ps_av_pool,
            tc.tile_pool(name="ps_op", bufs=1, space="PSUM") as ps_op_pool,
        ):
            qT = persist.tile([128, 2, S], bf16)     # [qd, m, s]
            kT = persist.tile([128, 2, S], bf16)
            vp = persist.tile([128, 16, NHL, HD], bf16)  # [s-part, st, h, col]
            bq_sb = persist.tile([128, 2, 1], f32)
            bk_sb = persist.tile([128, 2, 1], f32)
            bv_bc = persist.tile([128, G], f32)
            wo_pr = persist.tile([128, 2, H], bf16)
            ones64 = persist.tile([1, 64], bf16)
            warm = persist.tile([128, 512], bf16)

            wq_sb = persist.tile([128, 8, G], bf16)
            wk_sb = persist.tile([128, 8, G], bf16)
            wv_sb = persist.tile([128, 8, G], bf16)

            # warmup scratch is memset (no DMA dependency) so the PE can
            # start ramping its clock immediately
            nc.gpsimd.memset(warm, 0.0)

            # ---- input DMAs, roughly in order of first use ----
            nc.sync.dma_start(
                out=wk_sb, in_=wk_in.ap().rearrange("(t p) d -> p t d", p=128))
            nc.sync.dma_start(
                out=wq_sb, in_=wq_in.ap().rearrange("(t p) d -> p t d", p=128))
            # per-chunk x^T tiles; host pre-tiles x^T into exactly this
            # layout so each chunk is one contiguous 1MB DMA; chained so
            # chunk 0 lands first at full bandwidth
            xTc = [persist.tile([128, 8, 512], bf16, name=f"xT_{jc}")
                   for jc in range(4)]
            x_dmas = [nc.sync.dma_start(out=xTc[jc], in_=xt_in.ap()[jc])
                      for jc in range(4)]
            for jc in range(1, 4):
                add_dep_helper(x_dmas[jc].ins, x_dmas[jc - 1].ins,
                               reason="serialize x chunk loads")
            nc.sync.dma_start(
                out=wv_sb, in_=wv_in.ap().rearrange("(t p) d -> p t d", p=128))
            nc.sync.dma_start(
                out=bq_sb, in_=bq_in.ap().rearrange("(m p) o -> p m o", p=128))
            nc.sync.dma_start(
                out=bk_sb, in_=bk_in.ap().rearrange("(m p) o -> p m o", p=128))
            # broadcast bv along partitions (stride-0 partition AP)
            bv_ap = bass.AP(tensor=bv_in, offset=0, ap=[[0, 128], [1, G]])
            nc.gpsimd.dma_start(out=bv_bc, in_=bv_ap)
            # Wo as stacked head pairs: [two*64+p, pr, n]
            nc.sync.dma_start(
                out=wo_pr,
                in_=wo_in.ap().rearrange("(pr two) p n -> (two p) pr n", two=2))
            # ones columns of V'
            nc.gpsimd.memset(vp[:, :, :, 64:65], 1.0)
            nc.gpsimd.memset(ones64, 1.0)

            # ---- warmup: keep the PE busy through the DMA lead-in ----
            # ~20k column-cycles covers the ~10us before wk/x0 land
            # (0.65/1.2 GHz ramp clocks), so real work starts warm.
            for wi in range(40):
                ps_d = ps_op_pool.tile([128, 512], f32, tag="dummy",
                                       name=f"warm_{wi}")
                nc.tensor.matmul(ps_d, lhsT=warm[:, 0:128], rhs=warm,
                                 start=True, stop=True)

            # ---- projection building blocks (used as fillers) ----
            # alternate PSUM tags so back-to-back fillers land in
            # different banks and don't serialize on the DVE evacuation
            _ftag = ["dummy"]

            def next_ftag():
                _ftag[0] = "oproj" if _ftag[0] == "dummy" else "dummy"
                return _ftag[0]

            def qk_half(w_sb, b_sb, dst, jc, m, half, st):
                sl = slice(jc * 512, (jc + 1) * 512)
                if half == 0:
                    st["ps"] = ps_op_pool.tile(
                        [128, 512], f32, tag=next_ftag(),
                        name=f"psqk_{id(w_sb)}_{jc}_{m}")
                for ht in range(half * 4, half * 4 + 4):
                    nc.tensor.matmul(
                        st["ps"],
                        lhsT=w_sb[:, ht, m * 128:(m + 1) * 128],
                        rhs=xTc[jc][:, ht, :],
                        start=(ht == 0), stop=(ht == 7))
                if half == 1:
                    nc.vector.tensor_scalar_add(
                        dst[:, m, sl], st["ps"], b_sb[:, m, :])

            def v_unit(st16):
                ps_vt = ps_op_pool.tile([128, 512], f32, tag=next_ftag(),
                                        name=f"psv_{st16}")
                for ht in range(8):
                    nc.tensor.matmul(
                        ps_vt[:, 0:G],
                        lhsT=xTc[st16 // 4][:, ht,
                                            (st16 % 4) * 128:
                                            (st16 % 4 + 1) * 128],
                        rhs=wv_sb[:, ht, :],
                        start=(ht == 0), stop=(ht == 7))
                nc.vector.tensor_add(
                    vp[:, st16, :, 0:64],
                    ps_vt[:, 0:G].rearrange("p (h d) -> p h d", h=NHL),
                    bv_bc.rearrange("p (h d) -> p h d", h=NHL))

            # ---- filler queue: (deadline_slot, cycles, closure) ----
            fillers = []

            def add_qk(w_sb, b_sb, dst, jc, m, deadline):
                st = {}
                fillers.append((deadline, 2048, lambda: qk_half(
                    w_sb, b_sb, dst, jc, m, 0, st)))
                fillers.append((deadline, 2048, lambda: qk_half(
                    w_sb, b_sb, dst, jc, m, 1, st)))

            # V st: needed by AV(st) issued at slot st+1
            for st16 in range(16):
                fillers.append((st16 + 1, 2048,
                                lambda s=st16: v_unit(s)))
            # K m0 jc1-3: needed by S(slot 4*jc)
            for jc in range(1, 4):
                add_qk(wk_sb, bk_sb, kT, jc, 0, 4 * jc)
            # K m1: needed by S of unit 1 (slots 16+4*jc)
            for jc in range(4):
                add_qk(wk_sb, bk_sb, kT, jc, 1, 16 + 4 * jc)
            # Q c0 m1: needed at slot 16
            add_qk(wq_sb, bq_sb, qT, 0, 1, 16)
            # Q c1-3 m0/m1: needed at unit starts
            for qc in range(1, 4):
                for m in range(2):
                    add_qk(wq_sb, bq_sb, qT, qc, m, 32 * qc + 16 * m)
            fillers.sort(key=lambda f: f[0])
            total_fill = sum(f[1] for f in fillers)
            fill_issued = [0]

            def run_filler():
                _, cyc, fn = fillers.pop(0)
                fn()
                fill_issued[0] += cyc

            def dummy(n):
                ps_d = ps_op_pool.tile([128, 512], f32, tag="dummy")
                nc.tensor.matmul(ps_d[:, 0:n], lhsT=kT[:, 0, 0:128],
                                 rhs=qT[:, 0, 0:n], start=True, stop=True)

            def norm_head(outP, ps_av, hh, qc, mt):
                # evacuate PSUM right away to release the bank; run the
                # normalize chain from SBUF
                uout = tmpo_pool.tile([HD, 512], f32, tag="uout",
                                      name=f"uo_{qc}_{mt}_{hh}", bufs=4)
                nc.vector.tensor_copy(uout, ps_av)
                sums = sums_pool.tile([1, 512], f32, tag="sums",
                                      name=f"sm_{qc}_{mt}_{hh}")
                nc.vector.tensor_copy(sums, uout[64:65, :])
                recip = sums_pool.tile([1, 512], f32, tag="recip",
                                       name=f"rc_{qc}_{mt}_{hh}")
                nc.vector.reciprocal_approx_fast(out=recip, in_=sums)
                recip_bf = sums_pool.tile([1, 512], bf16, tag="recipb",
                                          name=f"rcb_{qc}_{mt}_{hh}")
                nc.vector.tensor_copy(recip_bf, recip)
                # broadcast along partitions: rank-1 outer product on
                # the PE (ones[1,64].T @ recip[1,512] -> [64,512])
                rbc = ps_op_pool.tile([64, 512], f32, tag="dummy",
                                      name=f"rb_{qc}_{mt}_{hh}")
                nc.tensor.matmul(rbc, lhsT=ones64, rhs=recip_bf,
                                 start=True, stop=True)
                nc.vector.tensor_mul(
                    outP[hh * 64:hh * 64 + 64, :], uout[0:64, :], rbc)

            def oproj_unit(qc, outPs, qt, tail=False):
                # out_proj for one q-tile (K=128 stacked pairs); at the
                # kernel tail the freed score slots double-buffer it
                osb = osb_pool.tile([128, H], bf16, tag="osb",
                                    name=f"osb_{qc}_{qt}")
                for ncx in range(2):
                    if tail:
                        ps_op = ps_s_pool.tile(
                            [128, 2, 512], f32, tag="s",
                            name=f"psot_{qc}_{qt}_{ncx}")[:, 0, :]
                    else:
                        ps_op = ps_op_pool.tile(
                            [128, 512], f32, tag="oproj",
                            name=f"pso_{qc}_{qt}_{ncx}")
                    for pr in range(2):
                        nc.tensor.matmul(
                            ps_op,
                            lhsT=outPs[pr][:, qt * 128:(qt + 1) * 128],
                            rhs=wo_pr[:, pr, ncx * 512:(ncx + 1) * 512],
                            start=(pr == 0), stop=(pr == 1))
                    nc.vector.tensor_copy(
                        osb[:, ncx * 512:(ncx + 1) * 512], ps_op)
                nc.sync.dma_start(
                    out=out_d.ap()[qc * 512 + qt * 128:
                                   qc * 512 + (qt + 1) * 128, :],
                    in_=osb)

            # ---- prologue: minimal pre-score critical path ----
            stp = {}
            qk_half(wk_sb, bk_sb, kT, 0, 0, 0, stp)
            qk_half(wk_sb, bk_sb, kT, 0, 0, 1, stp)
            stp = {}
            qk_half(wq_sb, bq_sb, qT, 0, 0, 0, stp)
            qk_half(wq_sb, bq_sb, qT, 0, 0, 1, stp)

            # ---- master attention loop: 8 units x 16 kt slots ----
            UNITS = [(qc, mt) for qc in range(4) for mt in range(2)]
            pend_av = None       # (attnT, ps_avs, qc, mt, kt) awaiting AV
            pend_norm = None     # (qc, mt, ps_avs) awaiting normalize
            pend_oproj = []      # oproj closures, drained one per window
            outP_by_qc = {}

            def issue_av(p):
                at_t, avs, p_qc, p_mt, p_kt = p
                for hh in range(2):
                    nc.tensor.matmul(
                        avs[hh],
                        lhsT=vp[:, p_kt, 2 * p_mt + hh, :],
                        rhs=at_t[:, hh, p_kt % 4, :],
                        start=(p_kt == 0), stop=(p_kt == 15))
                return (p_qc, p_mt, avs) if p_kt == 15 else None

            def do_norm(p_qc, p_mt, avs, last=False):
                outP = op_pool.tile([128, 512], bf16, tag="outP",
                                    name=f"outP_{p_qc}_{p_mt}")
                for hh in range(2):
                    norm_head(outP, avs[hh], hh, p_qc, p_mt)
                outP_by_qc.setdefault(p_qc, []).append(outP)
                if p_mt == 1 and not last:
                    pouts = outP_by_qc.pop(p_qc)
                    for qt in range(4):
                        pend_oproj.append(
                            lambda q=p_qc, o=pouts, t=qt:
                            oproj_unit(q, o, t))

            for s in range(128):
                u, kt = s // 16, s % 16
                qc, mt = UNITS[u]
                qsl = slice(qc * 512, (qc + 1) * 512)

                if kt == 0:
                    attnT = at_pool.tile([128, 2, 4, 512], bf16,
                                         tag="at", name=f"at_{qc}_{mt}")
                    ps_avs = [ps_av_pool.tile([HD, 512], f32, tag="av",
                                              name=f"av_{qc}_{mt}_{hh}")
                              for hh in range(2)]

                # forced fillers: everything whose deadline has arrived
                while fillers and fillers[0][0] <= s:
                    run_filler()

                # scores + exp for this slot
                ps_s = ps_s_pool.tile([128, 2, 512], f32, tag="s")
                for hh in range(2):
                    nc.tensor.matmul(
                        ps_s[:, hh, :],
                        lhsT=kT[hh * 64:hh * 64 + 64, mt,
                                kt * 128:(kt + 1) * 128],
                        rhs=qT[hh * 64:hh * 64 + 64, mt, qsl],
                        start=True, stop=True)
                nc.scalar.activation(
                    out=attnT[:, :, kt % 4, :], in_=ps_s, func=EXP)

                # lagged AV from the previous slot; when it closes a
                # unit (kt==15), queue that unit's normalize
                if pend_av is not None:
                    done = issue_av(pend_av)
                    if done is not None:
                        pend_norm = done
                pend_av = (attnT, ps_avs, qc, mt, kt)

                # normalize the unit whose AV stream just closed
                if pend_norm is not None and kt == 1:
                    do_norm(*pend_norm)
                    pend_norm = None

                # out_proj: one q-tile per 4-slot window
                if pend_oproj and kt % 4 == 2:
                    pend_oproj.pop(0)()

                # paced optional fillers: keep the stream carrying real
                # work end-to-end instead of front-loading
                while (fillers and
                       fill_issued[0] * 116 < total_fill * (s + 1)):
                    run_filler()

                if not fillers and not pend_oproj and kt % 4 == 3:
                    dummy(256)

            # ---- tail: AV(15) of last unit, final norm, oproj ----
            while pend_oproj:
                pend_oproj.pop(0)()
            p_qc, p_mt, avs = issue_av(pend_av)
            do_norm(p_qc, p_mt, avs, last=True)
            pouts = outP_by_qc.pop(3)
            for qt in range(4):
                oproj_unit(3, pouts, qt, tail=True)

    nc.compile()
    _CACHE["nc"] = nc
    return nc


def make_in_maps(x, Wq, bq, Wk, bk, Wv, bv, Wo):
    import ml_dtypes
    bf = ml_dtypes.bfloat16

    x = np.asarray(x, dtype=np.float32)
    Wq = np.asarray(Wq, dtype=np.float32)
    bq = np.asarray(bq, dtype=np.float32)
    Wk = np.asarray(Wk, dtype=np.float32)
    bk = np.asarray(bk, dtype=np.float32)
    Wv = np.asarray(Wv, dtype=np.float32)
    bv = np.asarray(bv, dtype=np.float32)
    Wo = np.asarray(Wo, dtype=np.float32)

    scale = np.float32(1.0 / 8.0)  # 1/sqrt(64)

    in_maps = []
    for core in range(N_CORES):
        b = core // 4
        g = core % 4
        cs = slice(g * G, (g + 1) * G)
        in_maps.append({
            "xt": np.ascontiguousarray(
                x[b].reshape(4, 512, 8, 128).transpose(0, 3, 2, 1)).astype(bf),
            "wq": np.ascontiguousarray(Wq[:, cs] * scale).astype(bf),
            "wk": np.ascontiguousarray(Wk[:, cs]).astype(bf),
            "wv": np.ascontiguousarray(Wv[:, cs]).astype(bf),
            "bq": np.ascontiguousarray((bq[cs] * scale).reshape(G, 1)),
            "bk": np.ascontiguousarray(bk[cs].reshape(G, 1)),
            "bv": np.ascontiguousarray(bv[cs]),
            "wo": np.ascontiguousarray(Wo[cs, :].reshape(NHL, 64, H)).astype(bf),
        })
    return in_maps


def kernel(x, Wq, bq, Wk, bk, Wv, bv, Wo, bo):
    from concourse.bass_utils import run_bass_kernel_spmd

    bo = np.asarray(bo, dtype=np.float32)
    nc = _build()
    in_maps = make_in_maps(x, Wq, bq, Wk, bk, Wv, bv, Wo)
    res = run_bass_kernel_spmd(nc, in_maps, core_ids=list(range(N_CORES)))

    out = np.empty((2, S, H), dtype=np.float32)
    for b in range(2):
        acc = res.results[4 * b]["out"].astype(np.float32)
        for g in range(1, 4):
            acc = acc + res.results[4 * b + g]["out"].astype(np.float32)
        out[b] = acc + bo
    return out


# revision 14
# speedup vs baseline: 1.0498x; 1.0317x over previous
"""Multi-head self-attention TRN2 Bass kernel.

Problem: x[2, 2048, 1024], 16 heads x 64 dim, fp32.
Sharding: 8 cores = 2 batches x 4 head-groups (4 heads each).
Each core computes its batch's partial output (its 4 heads through
QKV -> attention -> output projection rows); host sums the 4 partials
per batch and adds bo.

Single fully-pipelined stream (no separate projection phase):
  - warmup matmuls on a memset tile from t~0 keep the PE p-state/HAM
    clock ramping while the input DMAs land (~9-12us lead-in).
  - minimal prologue: kT m=0 seq-chunk 0 + qT chunk 0 m=0, then the
    attention master loop starts immediately (~13us vs ~49us before).
  - ALL remaining projection work (K m0 jc1-3, K m1, Q c0 m1, Q c1-3,
    V st0-15) runs as deadline-scheduled fillers inside the attention
    stream: forced just-in-time by data deadlines, plus linear pacing
    so the late (ACT-bound) units carry real work instead of dummies.
  - scores computed TRANSPOSED per head-pair (two K=64 matmuls); exp on
    ACT -> A^T bf16 rolling buffer; AV lagged one kt slot behind exp so
    the PE never waits on the ACT engine.
  - normalize via fast-reciprocal + rank-1 PE broadcast; out_proj
    (K=128 stacked head-pairs) enqueued as fillers into the next unit.
  - output partials DMA'd out as bf16 (halves output traffic; host
    accumulates in f32 and adds bo).
"""

import numpy as np

S = 2048          # sequence length per batch
H = 1024          # hidden
G = 256           # head-group width (4 heads x 64)
HD = 65           # V' columns per head (64 + ones)
NHL = 4           # heads per core
N_CORES = 8

_CACHE = {}


def _build():
    if "nc" in _CACHE:
        return _CACHE["nc"]

    import concourse.bass as bass
    import concourse.mybir as mybir
    import concourse.tile as tile
    from concourse import bacc
    from concourse.tile_rust import add_dep_helper

    f32 = mybir.dt.float32
    bf16 = mybir.dt.bfloat16
    EXP = mybir.ActivationFunctionType.Exp

    nc = bacc.Bacc("TRN2", target_bir_lowering=False, debug=False,
                   num_devices=N_CORES)

    xt_in = nc.dram_tensor("xt", [4, 128, 8, 512], bf16, kind="ExternalInput")
    wq_in = nc.dram_tensor("wq", [H, G], bf16, kind="ExternalInput")
    wk_in = nc.dram_tensor("wk", [H, G], bf16, kind="ExternalInput")
    wv_in = nc.dram_tensor("wv", [H, G], bf16, kind="ExternalInput")
    bq_in = nc.dram_tensor("bq", [G, 1], f32, kind="ExternalInput")
    bk_in = nc.dram_tensor("bk", [G, 1], f32, kind="ExternalInput")
    bv_in = nc.dram_tensor("bv", [G], f32, kind="ExternalInput")
    wo_in = nc.dram_tensor("wo", [NHL, 64, H], bf16, kind="ExternalInput")
    out_d = nc.dram_tensor("out", [S, H], bf16, kind="ExternalOutput")

    with tile.TileContext(nc) as tc:
        with (
            tc.tile_pool(name="persist", bufs=1) as persist,
            tc.tile_pool(name="at_roll", bufs=2) as at_pool,
            tc.tile_pool(name="outP", bufs=4) as op_pool,
            tc.tile_pool(name="tmpo", bufs=1) as tmpo_pool,
            tc.tile_pool(name="sums", bufs=4) as sums_pool,
            tc.tile_pool(name="osb", bufs=2) as osb_pool,
            tc.tile_pool(name="ps_s", bufs=2, space="PSUM") as ps_s_pool,
            tc.tile_pool(name="ps_av", bufs=2, space="PSUM") as ps_av_pool,
            tc.tile_pool(name="ps_op", bufs=1, space="PSUM") as ps_op_pool,
        ):
            qT = persist.tile([128, 2, S], bf16)     # [qd, m, s]
            kT = persist.tile([128, 2, S], bf16)
            vp = persist.tile([128, 16, NHL, HD], bf16)  # [s-part, st, h, col]
            bq_sb = persist.tile([128, 2, 1], f32)
            bk_sb = persist.tile([128, 2, 1], f32)
            bv_bc = persist.tile([128, G], f32)
            wo_pr = persist.tile([128, 2, H], bf16)
            ones64 = persist.tile([1, 64], bf16)
            warm = persist.tile([128, 512], bf16)

            wq_sb = persist.tile([128, 8, G], bf16)
            wk_sb = persist.tile([128, 8, G], bf16)
            wv_sb = persist.tile([128, 8, G], bf16)

            # warmup scratch is memset (no DMA dependency) so the PE can
            # start ramping its clock immediately
            nc.gpsimd.memset(warm, 0.0)

            # ---- input DMAs, roughly in order of first use ----
            nc.sync.dma_start(
                out=wk_sb, in_=wk_in.ap().rearrange("(t p) d -> p t d", p=128))
            nc.sync.dma_start(
                out=wq_sb, in_=wq_in.ap().rearrange("(t p) d -> p t d", p=128))
            # per-chunk x^T tiles; host pre-tiles x^T into exactly this
            # layout so each chunk is one contiguous 1MB DMA; chained so
            # chunk 0 lands first at full bandwidth
            xTc = [persist.tile([128, 8, 512], bf16, name=f"xT_{jc}")
                   for jc in range(4)]
            x_dmas = [nc.sync.dma_start(out=xTc[jc], in_=xt_in.ap()[jc])
                      for jc in range(4)]
            for jc in range(1, 4):
                add_dep_helper(x_dmas[jc].ins, x_dmas[jc - 1].ins,
                               reason="serialize x chunk loads")
            nc.sync.dma_start(
                out=wv_sb, in_=wv_in.ap().rearrange("(t p) d -> p t d", p=128))
            nc.sync.dma_start(
                out=bq_sb, in_=bq_in.ap().rearrange("(m p) o -> p m o", p=128))
            nc.sync.dma_start(
                out=bk_sb, in_=bk_in.ap().rearrange("(m p) o -> p m o", p=128))
            # broadcast bv along partitions (stride-0 partition AP)
            bv_ap = bass.AP(tensor=bv_in, offset=0, ap=[[0, 128], [1, G]])
            nc.gpsimd.dma_start(out=bv_bc, in_=bv_ap)
            # Wo as stacked head pairs: [two*64+p, pr, n]
            nc.sync.dma_start(
                out=wo_pr,
                in_=wo_in.ap().rearrange("(pr two) p n -> (two p) pr n", two=2))
            # ones columns of V'
            nc.gpsimd.memset(vp[:, :, :, 64:65], 1.0)
            nc.gpsimd.memset(ones64, 1.0)

            # ---- warmup: keep the PE busy through the DMA lead-in ----
            # sized to END when wk/x0 land (~12us): the PE runs at the
            # pre-HAM half clock (~0.9GHz) here, so ~4k column-cycles.
            for wi in range(8):
                ps_d = ps_op_pool.tile([128, 512], f32, tag="dummy",
                                       name=f"warm_{wi}")
                nc.tensor.matmul(ps_d, lhsT=warm[:, 0:128], rhs=warm,
                                 start=True, stop=True)

            # ---- projection building blocks (used as fillers) ----
            # alternate PSUM tags so back-to-back fillers land in
            # different banks and don't serialize on the DVE evacuation
            _ftag = ["dummy"]

            def next_ftag():
                _ftag[0] = "oproj" if _ftag[0] == "dummy" else "dummy"
                return _ftag[0]

            def qk_half(w_sb, b_sb, dst, jc, m, half, st):
                sl = slice(jc * 512, (jc + 1) * 512)
                if half == 0:
                    st["ps"] = ps_op_pool.tile(
                        [128, 512], f32, tag=next_ftag(),
                        name=f"psqk_{id(w_sb)}_{jc}_{m}")
                for ht in range(half * 4, half * 4 + 4):
                    nc.tensor.matmul(
                        st["ps"],
                        lhsT=w_sb[:, ht, m * 128:(m + 1) * 128],
                        rhs=xTc[jc][:, ht, :],
                        start=(ht == 0), stop=(ht == 7))
                if half == 1:
                    nc.vector.tensor_scalar_add(
                        dst[:, m, sl], st["ps"], b_sb[:, m, :])

            def v_unit(st16):
                ps_vt = ps_op_pool.tile([128, 512], f32, tag=next_ftag(),
                                        name=f"psv_{st16}")
                for ht in range(8):
                    nc.tensor.matmul(
                        ps_vt[:, 0:G],
                        lhsT=xTc[st16 // 4][:, ht,
                                            (st16 % 4) * 128:
                                            (st16 % 4 + 1) * 128],
                        rhs=wv_sb[:, ht, :],
                        start=(ht == 0), stop=(ht == 7))
                nc.vector.tensor_add(
                    vp[:, st16, :, 0:64],
                    ps_vt[:, 0:G].rearrange("p (h d) -> p h d", h=NHL),
                    bv_bc.rearrange("p (h d) -> p h d", h=NHL))

            # ---- filler queue: (deadline_slot, cycles, closure) ----
            fillers = []

            def add_qk(w_sb, b_sb, dst, jc, m, deadline):
                st = {}
                fillers.append((deadline, 2048, lambda: qk_half(
                    w_sb, b_sb, dst, jc, m, 0, st)))
                fillers.append((deadline, 2048, lambda: qk_half(
                    w_sb, b_sb, dst, jc, m, 1, st)))

            # V st: needed by AV(st) issued at slot st+1
            for st16 in range(16):
                fillers.append((st16 + 1, 2048,
                                lambda s=st16: v_unit(s)))
            # K m0 jc1-3: needed by S(slot 4*jc)
            for jc in range(1, 4):
                add_qk(wk_sb, bk_sb, kT, jc, 0, 4 * jc)
            # K m1: needed by S of unit 1 (slots 16+4*jc)
            for jc in range(4):
                add_qk(wk_sb, bk_sb, kT, jc, 1, 16 + 4 * jc)
            # Q c0 m1: needed at slot 16
            add_qk(wq_sb, bq_sb, qT, 0, 1, 16)
            # Q c1-3 m0/m1: needed at unit starts
            for qc in range(1, 4):
                for m in range(2):
                    add_qk(wq_sb, bq_sb, qT, qc, m, 32 * qc + 16 * m)
            fillers.sort(key=lambda f: f[0])
            total_fill = sum(f[1] for f in fillers)
            fill_issued = [0]

            def run_filler():
                _, cyc, fn = fillers.pop(0)
                fn()
                fill_issued[0] += cyc

            def dummy(n):
                ps_d = ps_op_pool.tile([128, 512], f32, tag="dummy")
                nc.tensor.matmul(ps_d[:, 0:n], lhsT=kT[:, 0, 0:128],
                                 rhs=qT[:, 0, 0:n], start=True, stop=True)

            def norm_head(outP, ps_av, hh, qc, mt):
                # evacuate PSUM right away to release the bank; run the
                # normalize chain from SBUF
                uout = tmpo_pool.tile([HD, 512], f32, tag="uout",
                                      name=f"uo_{qc}_{mt}_{hh}", bufs=4)
                nc.vector.tensor_copy(uout, ps_av)
                sums = sums_pool.tile([1, 512], f32, tag="sums",
                                      name=f"sm_{qc}_{mt}_{hh}")
                nc.vector.tensor_copy(sums, uout[64:65, :])
                recip = sums_pool.tile([1, 512], f32, tag="recip",
                                       name=f"rc_{qc}_{mt}_{hh}")
                nc.vector.reciprocal_approx_fast(out=recip, in_=sums)
                recip_bf = sums_pool.tile([1, 512], bf16, tag="recipb",
                                          name=f"rcb_{qc}_{mt}_{hh}")
                nc.vector.tensor_copy(recip_bf, recip)
                # broadcast along partitions: rank-1 outer product on
                # the PE (ones[1,64].T @ recip[1,512] -> [64,512])
                rbc = ps_op_pool.tile([64, 512], f32, tag="dummy",
                                      name=f"rb_{qc}_{mt}_{hh}")
                nc.tensor.matmul(rbc, lhsT=ones64, rhs=recip_bf,
                                 start=True, stop=True)
                nc.vector.tensor_mul(
                    outP[hh * 64:hh * 64 + 64, :], uout[0:64, :], rbc)

            def oproj_unit(qc, outPs, qt, tail=False):
                # out_proj for one q-tile (K=128 stacked pairs); at the
                # kernel tail the freed score slots double-buffer it and
                # each half is DMA'd as soon as it is evacuated
                osb = osb_pool.tile([128, H], bf16, tag="osb",
                                    name=f"osb_{qc}_{qt}")
                r0 = qc * 512 + qt * 128
                for ncx in range(2):
                    if tail:
                        ps_op = ps_s_pool.tile(
                            [128, 2, 512], f32, tag="s",
                            name=f"psot_{qc}_{qt}_{ncx}")[:, 0, :]
                    else:
                        ps_op = ps_op_pool.tile(
                            [128, 512], f32, tag="oproj",
                            name=f"pso_{qc}_{qt}_{ncx}")
                    for pr in range(2):
                        nc.tensor.matmul(
                            ps_op,
                            lhsT=outPs[pr][:, qt * 128:(qt + 1) * 128],
                            rhs=wo_pr[:, pr, ncx * 512:(ncx + 1) * 512],
                            start=(pr == 0), stop=(pr == 1))
                    nc.vector.tensor_copy(
                        osb[:, ncx * 512:(ncx + 1) * 512], ps_op)
                    if tail:
                        nc.sync.dma_start(
                            out=out_d.ap()[r0:r0 + 128,
                                           ncx * 512:(ncx + 1) * 512],
                            in_=osb[:, ncx * 512:(ncx + 1) * 512])
                if not tail:
                    nc.sync.dma_start(out=out_d.ap()[r0:r0 + 128, :],
                                      in_=osb)

            # ---- prologue: minimal pre-score critical path ----
            stp = {}
            qk_half(wk_sb, bk_sb, kT, 0, 0, 0, stp)
            qk_half(wk_sb, bk_sb, kT, 0, 0, 1, stp)
            stp = {}
            qk_half(wq_sb, bq_sb, qT, 0, 0, 0, stp)
            qk_half(wq_sb, bq_sb, qT, 0, 0, 1, stp)

            # ---- master attention loop: 8 units x 16 kt slots ----
            UNITS = [(qc, mt) for qc in range(4) for mt in range(2)]
            pend_av = None       # (attnT, ps_avs, qc, mt, kt) awaiting AV
            pend_norm = None     # (qc, mt, ps_avs) awaiting normalize
            pend_oproj = []      # oproj closures, drained one per window
            outP_by_qc = {}

            def issue_av(p):
                at_t, avs, p_qc, p_mt, p_kt = p
                for hh in range(2):
                    nc.tensor.matmul(
                        avs[hh],
                        lhsT=vp[:, p_kt, 2 * p_mt + hh, :],
                        rhs=at_t[:, hh, p_kt % 4, :],
                        start=(p_kt == 0), stop=(p_kt == 15))
                return (p_qc, p_mt, avs) if p_kt == 15 else None

            def do_norm(p_qc, p_mt, avs, norm_slot=0, last=False):
                outP = op_pool.tile([128, 512], bf16, tag="outP",
                                    name=f"outP_{p_qc}_{p_mt}")
                for hh in range(2):
                    norm_head(outP, avs[hh], hh, p_qc, p_mt)
                outP_by_qc.setdefault(p_qc, []).append(outP)
                if p_mt == 1 and not last:
                    pouts = outP_by_qc.pop(p_qc)
                    for qt in range(4):
                        # defer the drain so half the out_proj work lands
                        # in the following (ACT-paced) unit
                        pend_oproj.append(
                            (norm_slot + 8,
                             lambda q=p_qc, o=pouts, t=qt:
                             oproj_unit(q, o, t)))

            for s in range(128):
                u, kt = s // 16, s % 16
                qc, mt = UNITS[u]
                qsl = slice(qc * 512, (qc + 1) * 512)

                if kt == 0:
                    attnT = at_pool.tile([128, 2, 4, 512], bf16,
                                         tag="at", name=f"at_{qc}_{mt}")
                    ps_avs = [ps_av_pool.tile([HD, 512], f32, tag="av",
                                              name=f"av_{qc}_{mt}_{hh}")
                              for hh in range(2)]

                # forced fillers: everything whose deadline has arrived
                while fillers and fillers[0][0] <= s:
                    run_filler()

                # scores + exp for this slot
                ps_s = ps_s_pool.tile([128, 2, 512], f32, tag="s")
                for hh in range(2):
                    nc.tensor.matmul(
                        ps_s[:, hh, :],
                        lhsT=kT[hh * 64:hh * 64 + 64, mt,
                                kt * 128:(kt + 1) * 128],
                        rhs=qT[hh * 64:hh * 64 + 64, mt, qsl],
                        start=True, stop=True)
                nc.scalar.activation(
                    out=attnT[:, :, kt % 4, :], in_=ps_s, func=EXP)

                # lagged AV from the previous slot; when it closes a
                # unit (kt==15), queue that unit's normalize
                if pend_av is not None:
                    done = issue_av(pend_av)
                    if done is not None:
                        pend_norm = done
                pend_av = (attnT, ps_avs, qc, mt, kt)

                # normalize the unit whose AV stream just closed
                if pend_norm is not None and kt == 1:
                    p_qc, p_mt, p_avs2 = pend_norm
                    do_norm(p_qc, p_mt, p_avs2, norm_slot=s)
                    pend_norm = None

                # out_proj: one q-tile per 4-slot window once eligible
                if pend_oproj and pend_oproj[0][0] <= s and kt % 4 == 2:
                    pend_oproj.pop(0)[1]()

                # paced optional fillers: keep the stream carrying real
                # work end-to-end instead of front-loading
                while (fillers and
                       fill_issued[0] * 116 < total_fill * (s + 1)):
                    run_filler()

                if not fillers and not pend_oproj and kt % 4 == 3:
                    dummy(256)

            # ---- tail: AV(15) of last unit, final norm, oproj ----
            # dummies keep the PE dense (HAM clock at full speed) while
            # the last exp + normalize chain drains
            while pend_oproj:
                pend_oproj.pop(0)[1]()
            for _ in range(3):
                dummy(512)
            p_qc, p_mt, avs = issue_av(pend_av)
            do_norm(p_qc, p_mt, avs, last=True)
            for _ in range(3):
                dummy(512)
            pouts = outP_by_qc.pop(3)
            for qt in range(4):
                oproj_unit(3, pouts, qt, tail=True)

    nc.compile()
    _CACHE["nc"] = nc
    return nc


def make_in_maps(x, Wq, bq, Wk, bk, Wv, bv, Wo):
    import ml_dtypes
    bf = ml_dtypes.bfloat16

    x = np.asarray(x, dtype=np.float32)
    Wq = np.asarray(Wq, dtype=np.float32)
    bq = np.asarray(bq, dtype=np.float32)
    Wk = np.asarray(Wk, dtype=np.float32)
    bk = np.asarray(bk, dtype=np.float32)
    Wv = np.asarray(Wv, dtype=np.float32)
    bv = np.asarray(bv, dtype=np.float32)
    Wo = np.asarray(Wo, dtype=np.float32)

    scale = np.float32(1.0 / 8.0)  # 1/sqrt(64)

    in_maps = []
    for core in range(N_CORES):
        b = core // 4
        g = core % 4
        cs = slice(g * G, (g + 1) * G)
        in_maps.append({
            "xt": np.ascontiguousarray(
                x[b].reshape(4, 512, 8, 128).transpose(0, 3, 2, 1)).astype(bf),
            "wq": np.ascontiguousarray(Wq[:, cs] * scale).astype(bf),
            "wk": np.ascontiguousarray(Wk[:, cs]).astype(bf),
            "wv": np.ascontiguousarray(Wv[:, cs]).astype(bf),
            "bq": np.ascontiguousarray((bq[cs] * scale).reshape(G, 1)),
            "bk": np.ascontiguousarray(bk[cs].reshape(G, 1)),
            "bv": np.ascontiguousarray(bv[cs]),
            "wo": np.ascontiguousarray(Wo[cs, :].reshape(NHL, 64, H)).astype(bf),
        })
    return in_maps


def kernel(x, Wq, bq, Wk, bk, Wv, bv, Wo, bo):
    from concourse.bass_utils import run_bass_kernel_spmd

    bo = np.asarray(bo, dtype=np.float32)
    nc = _build()
    in_maps = make_in_maps(x, Wq, bq, Wk, bk, Wv, bv, Wo)
    res = run_bass_kernel_spmd(nc, in_maps, core_ids=list(range(N_CORES)))

    out = np.empty((2, S, H), dtype=np.float32)
    for b in range(2):
        acc = res.results[4 * b]["out"].astype(np.float32)
        for g in range(1, 4):
            acc = acc + res.results[4 * b + g]["out"].astype(np.float32)
        out[b] = acc + bo
    return out
